# revision 11
# baseline (speedup 1.0000x reference)
"""Causal self-attention on 8 Trainium2 NeuronCores.

Sharding: core c = (batch b = c//2) x (head-half h2 = c%2). Each core
computes, for its batch and its 8 heads (of 16): the QKV projection
(only its W_qkv columns), causal flash attention, and a *partial*
output projection against its 512 rows of W_out. The host sums the
two half partials per batch and adds b_out. No device collectives.

On-device layout (per core):
  xT   (1024, 2048)  x[b] transposed (host-side, free)
  Q^T/K^T (64, T) per head   -- from W-stationary matmuls (qkv^T comes
                                out with channel on partitions)
  V    (T, 64) per head, with a fused ones-column (65 cols) so the
       P@V matmul also emits the softmax denominator row.
  S^T  (k-part, q-free) blocks -> exp on ScalarE (scale=1/8 fused,
       causal mask added only on diagonal 128x128 blocks)
  O'^T (65, q) accumulated in PSUM over k-blocks; row 64 = denom.
  Normalize on VectorE with a GpSimd partition-broadcast reciprocal.
  Out-proj: lhsT = paired-head O^T chunks, rhs = W_out rows.

All matmuls run in float32r (TF32-like, 1 cyc/row at N>=256).
"""
import os
import sys

sys.path.insert(0, "/opt/trn_rl_repo")

import numpy as np

import concourse.bacc as bacc
import concourse.mybir as mybir
import concourse.tile as tile
from concourse.bass_utils import run_bass_kernel_spmd

B, T, C = 4, 2048, 1024
H = 16
HD = C // H              # 64
N_CORES = 8
HL = H // 2              # 8 local heads per core
CL = HL * HD             # 512 local channels
F32 = mybir.dt.float32
F32R = mybir.dt.float32r

QG = 1024                # q-group width in phase 2
NQG = T // QG            # 2
KB = 128                 # k-block
NKB = T // KB            # 16
TCH = 128                # t-chunk (tokens per matmul M)
NTCH = T // TCH          # 16
CCH = 128                # channel chunk (contraction tile)
NCCH = C // CCH          # 8

_cache = {}


def _build(dbg=False, reps=1):
    nc = bacc.Bacc("TRN2", target_bir_lowering=False, debug=False,
                   num_devices=N_CORES)

    xT = nc.dram_tensor("xT", [C, T], F32R, kind="ExternalInput")
    wqk = nc.dram_tensor("wqk", [C, 2 * CL], F32R, kind="ExternalInput")
    wv = nc.dram_tensor("wv", [C, CL], F32R, kind="ExternalInput")
    wout = nc.dram_tensor("wout", [CL, C], F32R, kind="ExternalInput")
    BF16 = mybir.dt.bfloat16
    mask = nc.dram_tensor("mask", [KB, KB], BF16, kind="ExternalInput")
    ident = nc.dram_tensor("ident", [KB, KB], BF16, kind="ExternalInput")
    y = nc.dram_tensor("y", [T, C], F32, kind="ExternalOutput")
    if dbg:
        d_qk0 = nc.dram_tensor("d_qk0", [128, T], F32, kind="ExternalOutput")
        d_qk4 = nc.dram_tensor("d_qk4", [128, T], F32, kind="ExternalOutput")
        d_vw0 = nc.dram_tensor("d_vw0", [128, HL * (HD + 1)], F32,
                               kind="ExternalOutput")
        d_p = nc.dram_tensor("d_p", [128, QG], F32, kind="ExternalOutput")
        d_rr = nc.dram_tensor("d_rr", [1, QG], F32, kind="ExternalOutput")
        d_rb = nc.dram_tensor("d_rb", [64, QG], F32, kind="ExternalOutput")
        d_ot0 = nc.dram_tensor("d_ot0", [128, T], F32, kind="ExternalOutput")

    with tile.TileContext(nc) as tc:
      for _rep in range(reps):
        with tc.tile_pool(name="persist", bufs=1) as pp:
            # persistent SBUF tiles
            qk = [pp.tile([128, T], F32R, tag=f"qk{j}", name=f"qk{j}") for j in range(8)]
            #   qk[0..3] = Q^T pairs (head 2j at part 0-63, 2j+1 at 64-127)
            #   qk[4..7] = K^T pairs
            vws = [pp.tile([128, HL * (HD + 1)], F32R, tag=f"vw{m}", name=f"vw{m}")
                   for m in range(NTCH)]      # V' tiles: (128t, 8*(64+1))
            ot = [pp.tile([128, T], F32R, tag=f"ot{j}", name=f"ot{j}") for j in range(4)]
            BF16 = mybir.dt.bfloat16
            msk = pp.tile([KB, KB], BF16, tag="msk", name="msk")
            nc.sync.dma_start(msk[:], mask[:])
            idn = pp.tile([KB, KB], BF16, tag="idn", name="idn")
            nc.sync.dma_start(idn[:], ident[:])

            # ---------------- Phase 1: QKV projection ----------------
            with (
                tc.tile_pool(name="p1", bufs=1) as p1,
                tc.tile_pool(name="p1w", bufs=2) as p1w,
                tc.tile_pool(name="ps_mm", bufs=4, space="PSUM") as ps_mm,
            ):
                wv_t = []
                for i in range(NCCH):
                    wv_i = p1.tile([128, CL], F32R, tag=f"wv{i}", name=f"wv{i}")
                    nc.sync.dma_start(
                        wv_i[:], wv[i * CCH:(i + 1) * CCH, :])
                    wv_t.append(wv_i)
                for half in range(2):
                    t0 = half * (T // 2)
                    xt = []
                    for i in range(NCCH):
                        xti = p1.tile([128, T // 2], F32R, tag=f"xt{i}", name=f"xt{i}")
                        nc.sync.dma_start(
                            xti[:], xT[i * CCH:(i + 1) * CCH, t0:t0 + T // 2])
                        xt.append(xti)

                    # Q^T / K^T: lhsT = wqk chunk (stationary, reused
                    # across the 2 t-groups of this half), rhs = xT.
                    for j in range(8):          # c' 128-chunks of [Q|K]
                        wc = []
                        for i in range(NCCH):
                            wci = p1w.tile([128, 128], F32R,
                                           tag=f"wc{i}", name=f"wc{i}")
                            nc.sync.dma_start(
                                wci[:], wqk[i * CCH:(i + 1) * CCH,
                                            j * 128:(j + 1) * 128])
                            wc.append(wci)
                        for tg in range(2):     # 512-wide t-groups
                            ps = ps_mm.tile([128, 512], F32, tag="mm", name="mm")
                            for i in range(NCCH):
                                nc.tensor.matmul(
                                    ps[:],
                                    wc[i][:],
                                    xt[i][:, tg * 512:(tg + 1) * 512],
                                    start=(i == 0), stop=(i == NCCH - 1))
                            dst = qk[j][:, t0 + tg * 512: t0 + (tg + 1) * 512]
                            nc.vector.tensor_copy(dst, ps[:])

                    # V: lhsT = xT t-chunk, rhs = wv columns.
                    for m in range(NTCH // 2):  # t-chunks in this half
                        ps = ps_mm.tile([128, CL], F32, tag="mm", name="mmv")
                        for i in range(NCCH):
                            nc.tensor.matmul(
                                ps[:],
                                xt[i][:, m * TCH:(m + 1) * TCH],
                                wv_t[i][:],
                                start=(i == 0), stop=(i == NCCH - 1))
                        vt = vws[half * (NTCH // 2) + m]
                        # scatter (h,d) -> (h, d | ones) layout
                        dst = vt[:].rearrange("p (h x) -> p h x", x=HD + 1)
                        nc.vector.tensor_copy(
                            dst[:, :, 0:HD],
                            ps[:].rearrange("p (h d) -> p h d", d=HD))
                        nc.vector.memset(dst[:, :, HD:HD + 1].bitcast(F32), 1.0)

            if dbg:
                nc.sync.dma_start(d_qk0[:], qk[0][:].bitcast(F32))
                nc.sync.dma_start(d_qk4[:], qk[4][:].bitcast(F32))
                nc.sync.dma_start(d_vw0[:], vws[0][:].bitcast(F32))

            # ---------------- Phase 2: attention ----------------
            with (
                tc.tile_pool(name="ps_s", bufs=2, space="PSUM") as ps_s,
                tc.tile_pool(name="ps_o", bufs=2, space="PSUM") as ps_o,
                tc.tile_pool(name="p2", bufs=3) as p2,
                tc.tile_pool(name="p2n", bufs=2) as p2n,
            ):
                for h in range(HL):
                    jp = h // 2               # pair index
                    pb = (h % 2) * 64         # partition base within pair
                    for g in range(NQG):
                        qlo = g * QG
                        nkb = (qlo + QG) // KB
                        o_ps = ps_o.tile([128, QG], F32, tag="ops", name="ops")

                        def emit_s(kb):
                            # S^T block: lhsT = K^T slice, rhs = Q^T; the
                            # causal mask on the diagonal is accumulated by
                            # a second (identity-weighted) matmul so the
                            # PSUM->exp chain stays PE->ACT with no DVE hop.
                            r0 = max(0, kb * KB - qlo)
                            s_ps = ps_s.tile([128, QG], F32, tag="sps",
                                             name="sps")
                            diag = kb * KB >= qlo
                            lhs = qk[4 + jp][pb:pb + 64,
                                             kb * KB:(kb + 1) * KB]
                            c0 = r0
                            while c0 < QG:
                                c1 = min(QG, (c0 // 512 + 1) * 512)
                                last = (not diag) or (c0 > r0) or (KB > c1 - c0)
                                nc.tensor.matmul(
                                    s_ps[:, c0:c1],
                                    lhs,
                                    qk[jp][pb:pb + 64, qlo + c0:qlo + c1],
                                    start=True,
                                    stop=(not diag) or (c0 != r0))
                                c0 = c1
                            if diag:
                                nc.tensor.matmul(
                                    s_ps[:, r0:r0 + KB], idn[:], msk[:],
                                    start=False, stop=True)
                            p_sb = p2.tile([128, QG], F32R, tag="p", name="p")
                            nc.scalar.activation(
                                p_sb[:, r0:], s_ps[:, r0:],
                                mybir.ActivationFunctionType.Exp,
                                scale=0.125)
                            if dbg and h == 0 and g == 0 and kb == 0:
                                nc.sync.dma_start(d_p[:],
                                                  p_sb[:].bitcast(F32))
                            return p_sb

                        def emit_pv(kb, p_sb):
                            # P@V' accumulate: out rows 0..64 (row 64 =
                            # softmax denominator via the ones column)
                            r0 = max(0, kb * KB - qlo)
                            lhv = vws[kb][:, h * (HD + 1):
                                          (h + 1) * (HD + 1)]
                            c0 = (r0 // 512) * 512
                            while c0 < QG:
                                c1 = min(QG, c0 + 512)
                                rs = max(c0, r0)
                                last_kb = min(nkb, (qlo + c1) // KB) - 1
                                nc.tensor.matmul(
                                    o_ps[0:HD + 1, rs:c1],
                                    lhv,
                                    p_sb[:, rs:c1],
                                    start=(kb == 0), stop=(kb == last_kb))
                                c0 = c1

                        prev = None
                        for kb in range(nkb):
                            p_sb = emit_s(kb)
                            if prev is not None:
                                emit_pv(*prev)
                            prev = (kb, p_sb)
                        emit_pv(*prev)
                        # normalize: recip of denom row, broadcast, mul.
                        # reciprocal is lane-locked (DVE), so it lands on
                        # partition 64; partition_broadcast reads physical
                        # partition 0, so DMA-hop the row down first.
                        rr = p2n.tile([65, QG], F32, tag="rr", name="rr")
                        nc.vector.reciprocal(rr[64:65, :], o_ps[HD:HD + 1, :])
                        rr0 = p2n.tile([1, QG], F32, tag="rr0", name="rr0")
                        nc.sync.dma_start(rr0[:], rr[64:65, :])
                        rb = p2n.tile([64, QG], F32, tag="rb", name="rb")
                        nc.gpsimd.partition_broadcast(rb[:], rr0[:])
                        if dbg and h == 0 and g == 0:
                            nc.sync.dma_start(d_rr[:], rr[64:65, :])
                            nc.sync.dma_start(d_rb[:], rb[:])
                        if pb == 0:
                            nc.vector.tensor_mul(
                                ot[jp][0:64, qlo:qlo + QG],
                                o_ps[0:HD, :], rb[:])
                        else:
                            os_ = p2n.tile([64, QG], F32R, tag="os", name="os")
                            nc.vector.tensor_mul(os_[:], o_ps[0:HD, :], rb[:])
                            nc.sync.dma_start(
                                ot[jp][64:128, qlo:qlo + QG], os_[:])

            if dbg:
                nc.sync.dma_start(d_ot0[:], ot[0][:].bitcast(F32))

            # ---------------- Phase 3: output projection ----------------
            with (
                tc.tile_pool(name="p3", bufs=2) as p3,
                tc.tile_pool(name="p3w", bufs=1) as p3w,
                tc.tile_pool(name="ps_mm", bufs=4, space="PSUM") as ps_mm,
            ):
                wo_t = []
                for j in range(4):
                    wj = p3w.tile([128, C], F32R, tag=f"wo{j}", name=f"wo{j}")
                    nc.sync.dma_start(wj[:], wout[j * 128:(j + 1) * 128, :])
                    wo_t.append(wj)
                for m in range(NTCH):
                    for n in range(2):
                        ps = ps_mm.tile([128, 512], F32, tag="mm", name="mm")
                        for j in range(4):
                            nc.tensor.matmul(
                                ps[:],
                                ot[j][:, m * TCH:(m + 1) * TCH],
                                wo_t[j][:, n * 512:(n + 1) * 512],
                                start=(j == 0), stop=(j == 3))
                        ysb = p3.tile([128, 512], F32, tag="y", name="y")
                        nc.scalar.copy(ysb[:], ps[:])
                        nc.sync.dma_start(
                            y[m * TCH:(m + 1) * TCH, n * 512:(n + 1) * 512],
                            ysb[:])

    nc.compile()
    return nc


def kernel(x, W_qkv, b_qkv, W_out, b_out, _trace=False):
    x = np.asarray(x, dtype=np.float32)
    W_qkv = np.asarray(W_qkv, dtype=np.float32)
    b_qkv = np.asarray(b_qkv, dtype=np.float32)
    W_out = np.asarray(W_out, dtype=np.float32)
    b_out = np.asarray(b_out, dtype=np.float32)

    # q/k biases would need device-side adds; this problem pins them to 0.
    assert not b_qkv[:2 * C].any(), "nonzero q/k bias unsupported"

    if "nc" not in _cache:
        _cache["nc"] = _build()
    nc = _cache["nc"]

    import ml_dtypes
    mask = np.where(
        np.arange(KB)[None, :] < np.arange(KB)[:, None], -1e30, 0.0
    ).astype(ml_dtypes.bfloat16)
    ident = np.eye(KB).astype(ml_dtypes.bfloat16)

    in_maps = []
    for c in range(N_CORES):
        b, h2 = c // 2, c % 2
        cols = slice(h2 * CL, (h2 + 1) * CL)
        in_maps.append({
            "xT": np.ascontiguousarray(x[b].T),
            "wqk": np.ascontiguousarray(
                np.concatenate([W_qkv[:, cols],
                                W_qkv[:, C:][:, cols]], axis=1)),
            "wv": np.ascontiguousarray(W_qkv[:, 2 * C:][:, cols]),
            "wout": np.ascontiguousarray(W_out[cols, :]),
            "mask": mask,
            "ident": ident,
        })

    kwargs = {}
    if _trace:
        kwargs = {"trace": True, "trace_cores": [0]}
    res = run_bass_kernel_spmd(nc, in_maps, core_ids=list(range(N_CORES)),
                               **kwargs)

    out = np.empty((B, T, C), dtype=np.float32)
    # v-bias passes through softmax as +b_v, so it folds into the output
    # projection; b_out likewise. Both are host-side adds on the partials.
    bias = b_qkv[2 * C:] @ W_out + b_out
    for b in range(B):
        out[b] = res.results[2 * b]["y"] + res.results[2 * b + 1]["y"] + bias
    if _trace:
        kernel.last_exec_ns = res.exec_time_ns
        kernel.last_trace = (res.instructions_and_trace or (None, None))[1]
    return out
